# revision 42
# baseline (speedup 1.0000x reference)
# Trainium2 Bass kernel v4 for nn_MetricLearningLoss1 (triplet loss with
# semi-hard negative mining over top-k-confidence-filtered embeddings).
#
# v2's pipeline with the on-device band-threshold phase replaced by a
# host-computed threshold (exactness on HW was verified by micro-kernels:
# fp16 dot matmuls match numpy fp32 to <3e-4, and the fp8 DoubleRow sqn
# matmul is bit-exact when every DR row pair is (value, 0) — the PE rounds
# each pair sum to ~fp16 otherwise).
#
# Strategy (8 NeuronCores, SPMD, no collectives):
#   host: top-k filter, sort by label, per-core anchor row-blocks; every
#         core holds the full fp16 embedding matrix as matmul rhs.
#         thr_a = (hardest-positive m-value) computed on host by replaying
#         device arithmetic over same-label pairs (~85k dots); shipped as
#         an exact f32 ScalarE bias (no quantization).
#   device (per core), per 128-anchor tile, per 512-col block:
#         2 fp16 matmuls -> p[a,j] = -2*a.e_j in PSUM.
#         ACT blocks: + rank-4 zero-interleaved DR matmul -> m = p + sqn_j;
#                     ScalarE Identity+bias(-thr-eps) -> x fp16.
#         DVE blocks: scalar_tensor_tensor x = (p - thr-eps) + SQN16.
#         min: DVE u16 tensor_tensor min chain over x blocks (positive fp16
#         bit patterns are order-isomorphic; negatives get the sign bit)
#         + one [128,512] u16 min reduce -> umin.
#   host: decode umin -> hard_neg; exact hard_pos from fp32 embeddings;
#         rows with no semi-hard candidate recomputed exactly.
import sys

sys.path.insert(0, "/opt/trn_rl_repo")

from contextlib import ExitStack

import numpy as np

# ---------------------------------------------------------------- constants
N_FULL, D = 32768, 256
TOPK = int(0.2 * N_FULL)  # 6553
NCORES = 8
MARGIN = np.float32(0.075)
# EPS covers the only device-vs-host divergence left: fp16-dot fp32
# accumulation order (<3e-4 measured on HW)
EPS = np.float32(1e-3)
PATCH_BAND = np.float32(1e-4)
THR_PAD = np.float32(3e4)  # pad-row threshold: x < 0 everywhere

FULL_DIMS = dict(n=TOPK, npad=896, ntiles=7, W=256, blk=512)
# block -> drain engine: blocks 0..ACT_SPLIT-1 drained by ScalarE activation
# (sqn added in PSUM via a rank-2 matmul; includes band blocks 0,1), blocks
# ACT_SPLIT.. drained by DVE scalar_tensor_tensor (sqn via SQN16 in1, no PE
# cost). u16 min chain: blocks 0..MIN_SPLIT-1 on DVE, the rest on GpSimd
# (GPSIMD cannot touch PSUM, but the min chain is SBUF-only).
ACT_SPLIT = 7
MIN_SPLIT = 13
PAD_SQ = -32000.0  # per SQ2 row at padding columns; forces x << 0 there


def _nblocks(dims):
    return (dims["n"] + dims["blk"] - 1) // dims["blk"]


# ---------------------------------------------------------------- builder
def build_nc(dims, repeat=1, act_split=ACT_SPLIT, min_split=MIN_SPLIT):
    import concourse.tile as tile
    from concourse import bacc, mybir

    n, npad, ntiles, W, blk = (
        dims["n"],
        dims["npad"],
        dims["ntiles"],
        dims["W"],
        dims["blk"],
    )
    NB = _nblocks(dims)
    assert npad == ntiles * 128

    nc = bacc.Bacc(
        "TRN2", target_bir_lowering=False, debug=False, num_devices=NCORES
    )
    f16, f32, u16 = mybir.dt.float16, mybir.dt.float32, mybir.dt.uint16
    u32 = mybir.dt.uint32
    Alu = mybir.AluOpType
    Act = mybir.ActivationFunctionType
    X = mybir.AxisListType.X

    L_d = nc.dram_tensor("L", [2, 128, npad], f16, kind="ExternalInput").ap()
    R_d = nc.dram_tensor("R", [NB, 128, 2 * blk], f16, kind="ExternalInput").ap()
    f8 = mybir.dt.float8e4
    DR = mybir.MatmulPerfMode.DoubleRow
    # zero-interleaved: 4 partitions x (sq-level row, zero row)
    SQ2_d = nc.dram_tensor("SQ2", [4, 2, NB * blk], f8, kind="ExternalInput").ap()
    ONES4_d = nc.dram_tensor("ONES4", [4, 2, 128], f8, kind="ExternalInput").ap()
    SQB_d = nc.dram_tensor(
        "SQB", [NB - act_split, 128, blk], f16, kind="ExternalInput"
    ).ap()
    NTHR_d = nc.dram_tensor("NTHR", [128, ntiles], f32, kind="ExternalInput").ap()
    umin_d = nc.dram_tensor(
        "umin_out", [ntiles, 128], u16, kind="ExternalOutput"
    ).ap()


    with tile.TileContext(nc) as tc, ExitStack() as ctx:
        rpool = ctx.enter_context(tc.tile_pool(name="r", bufs=1))
        lpool = ctx.enter_context(tc.tile_pool(name="l", bufs=1))
        sqpool = ctx.enter_context(tc.tile_pool(name="sq", bufs=1))
        psum = ctx.enter_context(tc.tile_pool(name="ps", bufs=8, space="PSUM"))
        xpool = ctx.enter_context(tc.tile_pool(name="x", bufs=8))
        mpool = ctx.enter_context(tc.tile_pool(name="minacc", bufs=5))
        spool = ctx.enter_context(tc.tile_pool(name="small", bufs=6))

        # persistent inputs; emission order = DMA priority. Transfers are
        # latency-floor-bound (~500ns each), so R channels are packed into
        # one [128, 2*blk] transfer per block and EQ into one tile.
        lt = []
        for c in range(2):
            t_ = lpool.tile([128, npad], f16, tag=f"l{c}")
            lt.append(t_)
        # sq2 carries sqn/2 in FOUR e4m3 levels, one per DR partition with a
        # zero row each (bit-exact on HW); one 107ns DoubleRow matmul per block
        sq2 = sqpool.tile([4, 2, NB * blk], f8, tag="sq2")
        ones2 = sqpool.tile([4, 2, 128], f8, tag="ones2")
        nthrall = sqpool.tile([128, ntiles], f32, tag="nthrall")

        rt = {}
        sqb = {}
        rtiles = []
        for b in range(NB):
            t_ = rpool.tile([128, 2 * blk], f16, tag=f"r{b}")
            rtiles.append(t_)
            rt[(0, b)] = t_[:, :blk]
            rt[(1, b)] = t_[:, blk:]
        # ship the minimal dep set first — L columns for tile 0, R0, sq2
        # chunk, nthr — then backfill
        nc.sync.dma_start(out=lt[0][:, :128], in_=L_d[0][:, :128])
        nc.sync.dma_start(out=lt[1][:, :128], in_=L_d[1][:, :128])
        nc.sync.dma_start(out=nthrall[:], in_=NTHR_d)
        nc.sync.dma_start(out=ones2[:], in_=ONES4_d)
        nc.sync.dma_start(out=rtiles[0][:], in_=R_d[0])
        nc.sync.dma_start(out=sq2[:, :, : 2 * blk], in_=SQ2_d[:, :, : 2 * blk])
        nc.sync.dma_start(out=rtiles[1][:], in_=R_d[1])
        nc.sync.dma_start(out=lt[0][:, 128:], in_=L_d[0][:, 128:])
        nc.sync.dma_start(out=lt[1][:, 128:], in_=L_d[1][:, 128:])
        hi = act_split * blk
        for b in range(2, NB):
            nc.sync.dma_start(out=rtiles[b][:], in_=R_d[b])
            if b == 3 and hi > 2 * blk:
                m6 = min(6 * blk, hi)
                nc.sync.dma_start(
                    out=sq2[:, :, 2 * blk : m6], in_=SQ2_d[:, :, 2 * blk : m6]
                )
            if b == 6 and hi > 6 * blk:
                nc.sync.dma_start(
                    out=sq2[:, :, 6 * blk : hi], in_=SQ2_d[:, :, 6 * blk : hi]
                )
            if b >= act_split:
                t_ = sqpool.tile([128, blk], f16, tag=f"sqb{b}")
                nc.sync.dma_start(out=t_[:], in_=SQB_d[b - act_split])
                sqb[b] = t_

        state = {}

        def bw(b):
            # real width of block b: the last block is ~80% padding
            return min(blk, n - b * blk)

        def matmuls(t, b):
            st = state[t]
            w = bw(b)
            p = psum.tile([128, blk], f32, tag="pm")
            tsl = slice(128 * t, 128 * (t + 1))
            nc.tensor.matmul(
                p[:, :w], lhsT=lt[0][:, tsl], rhs=rt[(0, b)][:, :w],
                start=True, stop=False,
            )
            last = b >= act_split
            nc.tensor.matmul(
                p[:, :w], lhsT=lt[1][:, tsl], rhs=rt[(1, b)][:, :w],
                start=False, stop=last,
            )
            if not last:
                nc.tensor.matmul(
                    p[:],
                    lhsT=ones2[:],
                    rhs=sq2[:, :, blk * b : blk * (b + 1)],
                    start=False,
                    stop=True,
                    perf_mode=DR,
                )
            st["pm"][b] = p

        def xop(t, b):
            st = state[t]
            w = bw(b)
            xb = xpool.tile([128, blk], f16, tag="xb")
            if b < act_split:
                nc.scalar.activation(
                    out=xb[:, :w],
                    in_=st["pm"][b][:, :w],
                    func=Act.Identity,
                    bias=st["nthr"][:],
                    scale=1.0,
                )
            else:
                nc.vector.scalar_tensor_tensor(
                    out=xb[:, :w],
                    in0=st["pm"][b][:, :w],
                    scalar=st["nthr"][:],
                    in1=sqb[b][:, :w],
                    op0=Alu.add,
                    op1=Alu.add,
                )
            del st["pm"][b]
            st["xblks"][b] = xb

        def minop(t, b):
            st = state[t]
            w = bw(b)
            xb = st["xblks"].pop(b)
            src = xb[:, :w].bitcast(u16)
            acc = st["minacc"]
            if not st.get("min_started"):
                pend = st.get("min_pending")
                if pend is None:
                    # defer: the first two blocks' mins fuse into one op
                    assert w == blk
                    st["min_pending"] = xb
                    return
                assert w == blk
                st["min_pending"] = None
                st["min_started"] = True
                nc.vector.tensor_tensor(
                    out=acc[:], in0=pend[:].bitcast(u16), in1=src, op=Alu.min
                )
                return
            nc.vector.tensor_tensor(
                out=acc[:, :w], in0=acc[:, :w], in1=src, op=Alu.min
            )

        def band_phase(t):
            # v2's band phase reduced to: state init (nthr is a host input
            # slice — zero instructions) + the first two blocks' matmuls,
            # preserving v2's emission order.
            st = dict(pm={}, xblks={})
            state[t] = st
            minacc = mpool.tile([128, blk], u16, tag="minacc")
            st["minacc"] = minacc
            st["min_started"] = False
            st["nthr"] = nthrall[:, t : t + 1]
            matmuls(t, 0)
            matmuls(t, 1)

        def finalize(t):
            st = state[t]
            # fold 512->256 at 2x-mode tt cost before the 1x-mode reduce
            acc = st["minacc"]
            nc.vector.tensor_tensor(
                out=acc[:, :256], in0=acc[:, :256], in1=acc[:, 256:], op=Alu.min
            )
            umin = spool.tile([128, 1], u16, tag="umin")
            nc.vector.tensor_reduce(
                out=umin[:], in_=acc[:, :256], axis=X, op=Alu.min
            )

            # outputs ride the SP HWDGE queue: issuing them on the
            # Activation queue punches 500ns holes into the pacing engine;
            # early tiles' outputs queue behind the input stream on SP, but
            # their buffers aren't reused until ~2 tiles later
            nc.sync.dma_start(out=umin_d[t], in_=umin[:, 0])
            del state[t]

        def main_rest(t):
            st = state[t]
            for b in range(2):
                xop(t, b)
                minop(t, b)
            order = list(range(2, NB))
            if t == ntiles - 1:
                # drain the DVE-stt blocks first so the kernel tail is owned
                # by the (lighter-loaded) Activation engine
                order = list(range(act_split, NB)) + list(range(2, act_split))
            for i, b in enumerate(order):
                matmuls(t, b)
                xop(t, b)
                minop(t, b)
                if i == 4 and t + 1 < ntiles:
                    band_phase(t + 1)
            finalize(t)

        NLEAD = 2

        def sweep():
            # The input stream paces the first ~3 tiles; interleave them
            # block-major so every arriving R block feeds NLEAD tiles of
            # engine work and the DMA latency hides completely.
            for t in range(NLEAD):
                band_phase(t)
            for b in range(NB):
                for t in range(NLEAD):
                    if b >= 2:
                        matmuls(t, b)
                    xop(t, b)
                    minop(t, b)
                if b == 8:
                    band_phase(NLEAD)
            for t in range(NLEAD):
                finalize(t)
            for t in range(NLEAD, ntiles):
                main_rest(t)

        if repeat == 1:
            sweep()
        else:
            with tc.For_i(0, repeat, 1):
                sweep()

    nc.compile()
    return nc


_NC_CACHE = {}


def _get_nc(key, dims):
    if key not in _NC_CACHE:
        _NC_CACHE[key] = build_nc(dims)
    return _NC_CACHE[key]


# ---------------------------------------------------------------- host side
def _e4m3_levels(q, nlev):
    """Greedy e4m3 residual decomposition with saturation clipping."""
    from ml_dtypes import float8_e4m3 as npf8

    levels = []
    r = q.astype(np.float32)
    for _ in range(nlev):
        s = np.clip(r, -240.0, 240.0).astype(npf8)
        levels.append(s)
        r = r - s.astype(np.float32)
    return levels


def host_prep(embeddings, tags, confidences, dims, act_split=ACT_SPLIT):
    from ml_dtypes import float8_e4m3 as npf8

    n, npad, ntiles, blk = dims["n"], dims["npad"], dims["ntiles"], dims["blk"]
    NB = _nblocks(dims)
    ncols = NB * blk
    conf = np.asarray(confidences, dtype=np.float32)
    order = np.argsort(-conf, kind="stable")[:n]
    emb = np.asarray(embeddings, dtype=np.float32)[order]
    labs = np.asarray(tags)[order]
    perm = np.argsort(labs, kind="stable")
    emb_s = np.ascontiguousarray(emb[perm], dtype=np.float32)
    labs_s = labs[perm]
    sqn = (emb_s**2).sum(axis=1, dtype=np.float32).astype(np.float32)
    counts = np.bincount(labs_s)
    valid = (counts[labs_s] >= 2) & (counts[labs_s] < n)

    # fp16 operands exactly as shipped to the device
    E16 = emb_s.astype(np.float16)
    L16 = (-2.0 * emb_s).astype(np.float16)
    EhiT = np.ascontiguousarray(E16.T)  # [256, n]

    sq_levels = _e4m3_levels(sqn * np.float32(0.5), 4)
    sq32dev = np.float32(2.0) * sum(lv.astype(np.float32) for lv in sq_levels)
    # per-column sqn value the DEVICE actually adds: DR-exact f32 for ACT
    # blocks, f16(sq32dev) via SQB for the stt blocks
    sqcol = sq32dev.copy()
    lo = act_split * blk
    if lo < n:
        sqcol[lo:] = sq32dev[lo:].astype(np.float16).astype(np.float32)

    # device-arithmetic hard-positive threshold + exact hard_pos, per label
    L32 = L16.astype(np.float32)
    E32 = E16.astype(np.float32)
    thr_raw = np.empty(n, np.float32)
    hp_exact = np.zeros(n, np.float32)
    ulabs = np.unique(labs_s)
    seg_starts = np.searchsorted(labs_s, ulabs, side="left")
    seg_ends = np.searchsorted(labs_s, ulabs, side="right")
    for s0, s1 in zip(seg_starts, seg_ends):
        k = s1 - s0
        if k == 1:
            thr_raw[s0] = L32[s0] @ E32[s0] + sqcol[s0]
            continue
        Mdev = L32[s0:s1] @ E32[s0:s1].T + sqcol[s0:s1][None, :]
        np.fill_diagonal(Mdev, -np.inf)
        thr_raw[s0:s1] = Mdev.max(axis=1)
        G = emb_s[s0:s1]
        D2 = (
            sqn[s0:s1][:, None]
            + sqn[s0:s1][None, :]
            - 2.0 * (G @ G.T).astype(np.float32)
        )
        np.fill_diagonal(D2, -np.inf)
        hp_exact[s0:s1] = np.sqrt(np.maximum(D2.max(axis=1), 0.0))

    # shared tensors (identical on every core)
    Rr = np.empty((2, 128, ncols), np.float16)
    Rr[0, :, :n] = EhiT[0:128]
    Rr[1, :, :n] = EhiT[128:256]
    Rr[:, :, n:] = 0
    R = np.ascontiguousarray(
        Rr.reshape(2, 128, NB, blk).transpose(2, 1, 0, 3).reshape(NB, 128, 2 * blk)
    )
    SQ2 = np.zeros((4, 2, ncols), npf8)
    for li in range(4):
        SQ2[li, 0, :n] = sq_levels[li]
        SQ2[li, 0, n:] = npf8(-240.0)  # pad cols: m ~ -1920, never a candidate
    ONES4 = np.zeros((4, 2, 128), npf8)
    ONES4[:, 0, :] = npf8(2.0)
    sqfull = np.full(ncols, -1920.0, np.float32)  # pad cols as in SQ2
    sqfull[:n] = sq32dev
    sq16 = sqfull[lo:].astype(np.float16)
    SQB = np.ascontiguousarray(
        np.broadcast_to(sq16, (128, ncols - lo))
        .reshape(128, NB - act_split, blk)
        .transpose(1, 0, 2)
    )

    starts = [round(k * n / NCORES) for k in range(NCORES + 1)]
    cores, in_maps = [], []
    nthr_all = -(thr_raw + EPS)
    for k in range(NCORES):
        a0, a1 = starts[k], starts[k + 1]
        cnt = a1 - a0
        b = np.zeros((npad, D), np.float32)
        b[:cnt] = -2.0 * emb_s[a0:a1]
        bhiT = b.T.astype(np.float16)
        L = np.stack([bhiT[0:128], bhiT[128:256]])
        nthr_c = np.full(npad, -THR_PAD, np.float32)
        nthr_c[:cnt] = nthr_all[a0:a1]
        NTHR = np.ascontiguousarray(nthr_c.reshape(ntiles, 128).T)
        cores.append(dict(a0=a0, cnt=cnt))
        in_maps.append(
            {"L": L, "R": R, "SQ2": SQ2, "ONES4": ONES4, "SQB": SQB, "NTHR": NTHR}
        )
    return (
        dict(
            emb_s=emb_s,
            labs_s=labs_s,
            sqn=sqn,
            valid=valid,
            cores=cores,
            n=n,
            hp=hp_exact,
            nthr=nthr_all,
        ),
        in_maps,
    )


def host_decode(prep, outs):
    n = prep["n"]
    emb_s, labs_s, sqn, valid, hp, nthr = (
        prep["emb_s"],
        prep["labs_s"],
        prep["sqn"],
        prep["valid"],
        prep["hp"],
        prep["nthr"],
    )
    terms = np.zeros(n, np.float32)
    patch_rows = []
    for k, core in enumerate(prep["cores"]):
        umin_v = outs[k].reshape(-1)
        a0, cnt = core["a0"], core["cnt"]
        xstar = umin_v[:cnt].view(np.float16).astype(np.float32)
        g = a0 + np.arange(cnt)
        vmask = valid[g]
        suspicious = ~(xstar > PATCH_BAND)
        for i in np.nonzero(vmask & suspicious)[0]:
            patch_rows.append(a0 + int(i))
        ok = vmask & ~suspicious
        idx = np.nonzero(ok)[0]
        if idx.size == 0:
            continue
        gg = g[idx]
        mstar = xstar[idx] - nthr[gg]  # x + thr + eps
        hn = np.sqrt(np.maximum(mstar + sqn[gg], 0.0), dtype=np.float32)
        terms[gg] = np.maximum(hp[gg] - hn + MARGIN, np.float32(0.0))

    patch_rows = sorted(set(patch_rows))
    if patch_rows:
        rows = np.array(patch_rows, np.int64)
        sq_rows = (
            sqn[rows][:, None]
            + sqn[None, :]
            - 2.0 * (emb_s[rows] @ emb_s.T).astype(np.float32)
        ).astype(np.float32)
        dist = np.sqrt(np.maximum(sq_rows, 0.0), dtype=np.float32)
        for ridx, gi in enumerate(rows):
            same = labs_s == labs_s[gi]
            pos = same.copy()
            pos[gi] = False
            neg = ~same
            if not pos.any() or not neg.any():
                terms[gi] = 0.0
                continue
            drow = dist[ridx]
            hard_pos = drow[pos].max()
            neg_min = drow[neg].min()
            shn = drow[neg & (drow > hard_pos)]
            hard_neg = shn.min() if shn.size else neg_min
            terms[gi] = max(hard_pos - hard_neg + MARGIN, np.float32(0.0))

    cnt_valid = valid.sum()
    if cnt_valid > 0:
        return np.float32(terms.sum(dtype=np.float32) / max(cnt_valid, 1))
    return np.float32(0.0)


# ---------------------------------------------------------------- entry
def kernel(embeddings, tags, confidences):
    from concourse.bass_utils import run_bass_kernel_spmd

    dims = FULL_DIMS
    nc = _get_nc("full", dims)
    prep, in_maps = host_prep(embeddings, tags, confidences, dims)
    res = run_bass_kernel_spmd(nc, in_maps, list(range(NCORES)))
    outs = [
        np.ascontiguousarray(res.results[k]["umin_out"]).astype(np.uint16)
        for k in range(NCORES)
    ]
    loss = host_decode(prep, outs)
    return np.array(loss, dtype=np.float32)


# revision 43
# speedup vs baseline: 1.0645x; 1.0645x over previous
# Trainium2 Bass kernel v4 for nn_MetricLearningLoss1 (triplet loss with
# semi-hard negative mining over top-k-confidence-filtered embeddings).
#
# v2's pipeline with the on-device band-threshold phase replaced by a
# host-computed threshold (exactness on HW was verified by micro-kernels:
# fp16 dot matmuls match numpy fp32 to <3e-4, and the fp8 DoubleRow sqn
# matmul is bit-exact when every DR row pair is (value, 0) — the PE rounds
# each pair sum to ~fp16 otherwise).
#
# Strategy (8 NeuronCores, SPMD, no collectives):
#   host: top-k filter, sort by label, per-core anchor row-blocks; every
#         core holds the full fp16 embedding matrix as matmul rhs.
#         thr_a = (hardest-positive m-value) computed on host by replaying
#         device arithmetic over same-label pairs (~85k dots); shipped as
#         an exact f32 ScalarE bias (no quantization).
#   device (per core), per 128-anchor tile, per 512-col block:
#         2 fp16 matmuls -> p[a,j] = -2*a.e_j in PSUM.
#         ACT blocks: + rank-4 zero-interleaved DR matmul -> m = p + sqn_j;
#                     ScalarE Identity+bias(-thr-eps) -> x fp16.
#         DVE blocks: scalar_tensor_tensor x = (p - thr-eps) + SQN16.
#         min: DVE u16 tensor_tensor min chain over x blocks (positive fp16
#         bit patterns are order-isomorphic; negatives get the sign bit)
#         + one [128,512] u16 min reduce -> umin.
#   host: decode umin -> hard_neg; exact hard_pos from fp32 embeddings;
#         rows with no semi-hard candidate recomputed exactly.
import sys

sys.path.insert(0, "/opt/trn_rl_repo")

from contextlib import ExitStack

import numpy as np

# ---------------------------------------------------------------- constants
N_FULL, D = 32768, 256
TOPK = int(0.2 * N_FULL)  # 6553
NCORES = 8
MARGIN = np.float32(0.075)
# EPS covers the only device-vs-host divergence left: fp16-dot fp32
# accumulation order (<3e-4 measured on HW)
EPS = np.float32(1e-3)
PATCH_BAND = np.float32(1e-4)
THR_PAD = np.float32(3e4)  # pad-row threshold: x < 0 everywhere

FULL_DIMS = dict(n=TOPK, npad=896, ntiles=7, W=256, blk=512)
# block -> drain engine: blocks 0..ACT_SPLIT-1 drained by ScalarE activation
# (sqn added in PSUM via a rank-2 matmul; includes band blocks 0,1), blocks
# ACT_SPLIT.. drained by DVE scalar_tensor_tensor (sqn via SQN16 in1, no PE
# cost). u16 min chain: blocks 0..MIN_SPLIT-1 on DVE, the rest on GpSimd
# (GPSIMD cannot touch PSUM, but the min chain is SBUF-only).
ACT_SPLIT = 8
MIN_SPLIT = 13
PAD_SQ = -32000.0  # per SQ2 row at padding columns; forces x << 0 there


def _nblocks(dims):
    return (dims["n"] + dims["blk"] - 1) // dims["blk"]


# ---------------------------------------------------------------- builder
def build_nc(dims, repeat=1, act_split=ACT_SPLIT, min_split=MIN_SPLIT):
    import concourse.tile as tile
    from concourse import bacc, mybir

    n, npad, ntiles, W, blk = (
        dims["n"],
        dims["npad"],
        dims["ntiles"],
        dims["W"],
        dims["blk"],
    )
    NB = _nblocks(dims)
    assert npad == ntiles * 128

    nc = bacc.Bacc(
        "TRN2", target_bir_lowering=False, debug=False, num_devices=NCORES
    )
    f16, f32, u16 = mybir.dt.float16, mybir.dt.float32, mybir.dt.uint16
    u32 = mybir.dt.uint32
    Alu = mybir.AluOpType
    Act = mybir.ActivationFunctionType
    X = mybir.AxisListType.X

    L_d = nc.dram_tensor("L", [2, 128, npad], f16, kind="ExternalInput").ap()
    R_d = nc.dram_tensor("R", [NB, 128, 2 * blk], f16, kind="ExternalInput").ap()
    f8 = mybir.dt.float8e4
    DR = mybir.MatmulPerfMode.DoubleRow
    # zero-interleaved: 4 partitions x (sq-level row, zero row)
    SQ2_d = nc.dram_tensor("SQ2", [4, 2, NB * blk], f8, kind="ExternalInput").ap()
    ONES4_d = nc.dram_tensor("ONES4", [4, 2, 128], f8, kind="ExternalInput").ap()
    SQB_d = nc.dram_tensor(
        "SQB", [NB - act_split, 128, blk], f16, kind="ExternalInput"
    ).ap()
    NTHR_d = nc.dram_tensor("NTHR", [128, ntiles], f32, kind="ExternalInput").ap()
    umin_d = nc.dram_tensor(
        "umin_out", [ntiles, 128], u16, kind="ExternalOutput"
    ).ap()


    with tile.TileContext(nc) as tc, ExitStack() as ctx:
        rpool = ctx.enter_context(tc.tile_pool(name="r", bufs=1))
        lpool = ctx.enter_context(tc.tile_pool(name="l", bufs=1))
        sqpool = ctx.enter_context(tc.tile_pool(name="sq", bufs=1))
        psum = ctx.enter_context(tc.tile_pool(name="ps", bufs=8, space="PSUM"))
        xpool = ctx.enter_context(tc.tile_pool(name="x", bufs=8))
        mpool = ctx.enter_context(tc.tile_pool(name="minacc", bufs=5))
        spool = ctx.enter_context(tc.tile_pool(name="small", bufs=6))

        # persistent inputs; emission order = DMA priority. Transfers are
        # latency-floor-bound (~500ns each), so R channels are packed into
        # one [128, 2*blk] transfer per block and EQ into one tile.
        lt = []
        for c in range(2):
            t_ = lpool.tile([128, npad], f16, tag=f"l{c}")
            lt.append(t_)
        # sq2 carries sqn/2 in FOUR e4m3 levels, one per DR partition with a
        # zero row each (bit-exact on HW); one 107ns DoubleRow matmul per block
        sq2 = sqpool.tile([4, 2, NB * blk], f8, tag="sq2")
        ones2 = sqpool.tile([4, 2, 128], f8, tag="ones2")
        nthrall = sqpool.tile([128, ntiles], f32, tag="nthrall")

        rt = {}
        sqb = {}
        rtiles = []
        for b in range(NB):
            t_ = rpool.tile([128, 2 * blk], f16, tag=f"r{b}")
            rtiles.append(t_)
            rt[(0, b)] = t_[:, :blk]
            rt[(1, b)] = t_[:, blk:]
        # ship the minimal dep set first — L columns for tile 0, R0, sq2
        # chunk, nthr — then backfill
        nc.sync.dma_start(out=lt[0][:, :128], in_=L_d[0][:, :128])
        nc.sync.dma_start(out=lt[1][:, :128], in_=L_d[1][:, :128])
        nc.sync.dma_start(out=nthrall[:], in_=NTHR_d)
        nc.sync.dma_start(out=ones2[:], in_=ONES4_d)
        nc.sync.dma_start(out=rtiles[0][:], in_=R_d[0])
        nc.sync.dma_start(out=sq2[:, :, : 2 * blk], in_=SQ2_d[:, :, : 2 * blk])
        nc.sync.dma_start(out=rtiles[1][:], in_=R_d[1])
        nc.sync.dma_start(out=lt[0][:, 128:], in_=L_d[0][:, 128:])
        nc.sync.dma_start(out=lt[1][:, 128:], in_=L_d[1][:, 128:])
        hi = act_split * blk
        for b in range(2, NB):
            nc.sync.dma_start(out=rtiles[b][:], in_=R_d[b])
            if b == 3 and hi > 2 * blk:
                m6 = min(6 * blk, hi)
                nc.sync.dma_start(
                    out=sq2[:, :, 2 * blk : m6], in_=SQ2_d[:, :, 2 * blk : m6]
                )
            if b == 6 and hi > 6 * blk:
                nc.sync.dma_start(
                    out=sq2[:, :, 6 * blk : hi], in_=SQ2_d[:, :, 6 * blk : hi]
                )
            if b >= act_split:
                t_ = sqpool.tile([128, blk], f16, tag=f"sqb{b}")
                nc.sync.dma_start(out=t_[:], in_=SQB_d[b - act_split])
                sqb[b] = t_

        state = {}

        def bw(b):
            # real width of block b: the last block is ~80% padding
            return min(blk, n - b * blk)

        def matmuls(t, b):
            st = state[t]
            w = bw(b)
            p = psum.tile([128, blk], f32, tag="pm")
            tsl = slice(128 * t, 128 * (t + 1))
            nc.tensor.matmul(
                p[:, :w], lhsT=lt[0][:, tsl], rhs=rt[(0, b)][:, :w],
                start=True, stop=False,
            )
            last = b >= act_split
            nc.tensor.matmul(
                p[:, :w], lhsT=lt[1][:, tsl], rhs=rt[(1, b)][:, :w],
                start=False, stop=last,
            )
            if not last:
                nc.tensor.matmul(
                    p[:],
                    lhsT=ones2[:],
                    rhs=sq2[:, :, blk * b : blk * (b + 1)],
                    start=False,
                    stop=True,
                    perf_mode=DR,
                )
            st["pm"][b] = p

        def xop(t, b):
            st = state[t]
            w = bw(b)
            xb = xpool.tile([128, blk], f16, tag="xb")
            if b < act_split:
                nc.scalar.activation(
                    out=xb[:, :w],
                    in_=st["pm"][b][:, :w],
                    func=Act.Identity,
                    bias=st["nthr"][:],
                    scale=1.0,
                )
            else:
                nc.vector.scalar_tensor_tensor(
                    out=xb[:, :w],
                    in0=st["pm"][b][:, :w],
                    scalar=st["nthr"][:],
                    in1=sqb[b][:, :w],
                    op0=Alu.add,
                    op1=Alu.add,
                )
            del st["pm"][b]
            st["xblks"][b] = xb

        def minop(t, b):
            st = state[t]
            w = bw(b)
            xb = st["xblks"].pop(b)
            src = xb[:, :w].bitcast(u16)
            acc = st["minacc"]
            if not st.get("min_started"):
                pend = st.get("min_pending")
                if pend is None:
                    # defer: the first two blocks' mins fuse into one op
                    assert w == blk
                    st["min_pending"] = xb
                    return
                assert w == blk
                st["min_pending"] = None
                st["min_started"] = True
                nc.vector.tensor_tensor(
                    out=acc[:], in0=pend[:].bitcast(u16), in1=src, op=Alu.min
                )
                return
            nc.vector.tensor_tensor(
                out=acc[:, :w], in0=acc[:, :w], in1=src, op=Alu.min
            )

        def band_phase(t):
            # v2's band phase reduced to: state init (nthr is a host input
            # slice — zero instructions) + the first two blocks' matmuls,
            # preserving v2's emission order.
            st = dict(pm={}, xblks={})
            state[t] = st
            minacc = mpool.tile([128, blk], u16, tag="minacc")
            st["minacc"] = minacc
            st["min_started"] = False
            st["nthr"] = nthrall[:, t : t + 1]
            matmuls(t, 0)
            matmuls(t, 1)

        def finalize(t):
            st = state[t]
            # fold 512->256 at 2x-mode tt cost before the 1x-mode reduce
            acc = st["minacc"]
            nc.vector.tensor_tensor(
                out=acc[:, :256], in0=acc[:, :256], in1=acc[:, 256:], op=Alu.min
            )
            umin = spool.tile([128, 1], u16, tag="umin")
            nc.vector.tensor_reduce(
                out=umin[:], in_=acc[:, :256], axis=X, op=Alu.min
            )

            # outputs ride the SP HWDGE queue: issuing them on the
            # Activation queue punches 500ns holes into the pacing engine;
            # early tiles' outputs queue behind the input stream on SP, but
            # their buffers aren't reused until ~2 tiles later
            nc.sync.dma_start(out=umin_d[t], in_=umin[:, 0])
            del state[t]

        def main_rest(t):
            st = state[t]
            for b in range(2):
                xop(t, b)
                minop(t, b)
            order = list(range(2, NB))
            if t == ntiles - 1:
                # drain the DVE-stt blocks first so the kernel tail is owned
                # by the (lighter-loaded) Activation engine
                order = list(range(act_split, NB)) + list(range(2, act_split))
            for i, b in enumerate(order):
                matmuls(t, b)
                xop(t, b)
                minop(t, b)
                if i == 4 and t + 1 < ntiles:
                    band_phase(t + 1)
            finalize(t)

        NLEAD = 2

        def sweep():
            # The input stream paces the first ~3 tiles; interleave them
            # block-major so every arriving R block feeds NLEAD tiles of
            # engine work and the DMA latency hides completely.
            for t in range(NLEAD):
                band_phase(t)
            for b in range(NB):
                for t in range(NLEAD):
                    if b >= 2:
                        matmuls(t, b)
                    xop(t, b)
                    minop(t, b)
                if b == 8:
                    band_phase(NLEAD)
            for t in range(NLEAD):
                finalize(t)
            for t in range(NLEAD, ntiles):
                main_rest(t)

        if repeat == 1:
            sweep()
        else:
            with tc.For_i(0, repeat, 1):
                sweep()

    nc.compile()
    return nc


_NC_CACHE = {}


def _get_nc(key, dims):
    if key not in _NC_CACHE:
        _NC_CACHE[key] = build_nc(dims)
    return _NC_CACHE[key]


# ---------------------------------------------------------------- host side
def _e4m3_levels(q, nlev):
    """Greedy e4m3 residual decomposition with saturation clipping."""
    from ml_dtypes import float8_e4m3 as npf8

    levels = []
    r = q.astype(np.float32)
    for _ in range(nlev):
        s = np.clip(r, -240.0, 240.0).astype(npf8)
        levels.append(s)
        r = r - s.astype(np.float32)
    return levels


def host_prep(embeddings, tags, confidences, dims, act_split=ACT_SPLIT):
    from ml_dtypes import float8_e4m3 as npf8

    n, npad, ntiles, blk = dims["n"], dims["npad"], dims["ntiles"], dims["blk"]
    NB = _nblocks(dims)
    ncols = NB * blk
    conf = np.asarray(confidences, dtype=np.float32)
    order = np.argsort(-conf, kind="stable")[:n]
    emb = np.asarray(embeddings, dtype=np.float32)[order]
    labs = np.asarray(tags)[order]
    perm = np.argsort(labs, kind="stable")
    emb_s = np.ascontiguousarray(emb[perm], dtype=np.float32)
    labs_s = labs[perm]
    sqn = (emb_s**2).sum(axis=1, dtype=np.float32).astype(np.float32)
    counts = np.bincount(labs_s)
    valid = (counts[labs_s] >= 2) & (counts[labs_s] < n)

    # fp16 operands exactly as shipped to the device
    E16 = emb_s.astype(np.float16)
    L16 = (-2.0 * emb_s).astype(np.float16)
    EhiT = np.ascontiguousarray(E16.T)  # [256, n]

    sq_levels = _e4m3_levels(sqn * np.float32(0.5), 4)
    sq32dev = np.float32(2.0) * sum(lv.astype(np.float32) for lv in sq_levels)
    # per-column sqn value the DEVICE actually adds: DR-exact f32 for ACT
    # blocks, f16(sq32dev) via SQB for the stt blocks
    sqcol = sq32dev.copy()
    lo = act_split * blk
    if lo < n:
        sqcol[lo:] = sq32dev[lo:].astype(np.float16).astype(np.float32)

    # device-arithmetic hard-positive threshold + exact hard_pos, per label
    L32 = L16.astype(np.float32)
    E32 = E16.astype(np.float32)
    thr_raw = np.empty(n, np.float32)
    hp_exact = np.zeros(n, np.float32)
    ulabs = np.unique(labs_s)
    seg_starts = np.searchsorted(labs_s, ulabs, side="left")
    seg_ends = np.searchsorted(labs_s, ulabs, side="right")
    for s0, s1 in zip(seg_starts, seg_ends):
        k = s1 - s0
        if k == 1:
            thr_raw[s0] = L32[s0] @ E32[s0] + sqcol[s0]
            continue
        Mdev = L32[s0:s1] @ E32[s0:s1].T + sqcol[s0:s1][None, :]
        np.fill_diagonal(Mdev, -np.inf)
        thr_raw[s0:s1] = Mdev.max(axis=1)
        G = emb_s[s0:s1]
        D2 = (
            sqn[s0:s1][:, None]
            + sqn[s0:s1][None, :]
            - 2.0 * (G @ G.T).astype(np.float32)
        )
        np.fill_diagonal(D2, -np.inf)
        hp_exact[s0:s1] = np.sqrt(np.maximum(D2.max(axis=1), 0.0))

    # shared tensors (identical on every core)
    Rr = np.empty((2, 128, ncols), np.float16)
    Rr[0, :, :n] = EhiT[0:128]
    Rr[1, :, :n] = EhiT[128:256]
    Rr[:, :, n:] = 0
    R = np.ascontiguousarray(
        Rr.reshape(2, 128, NB, blk).transpose(2, 1, 0, 3).reshape(NB, 128, 2 * blk)
    )
    SQ2 = np.zeros((4, 2, ncols), npf8)
    for li in range(4):
        SQ2[li, 0, :n] = sq_levels[li]
        SQ2[li, 0, n:] = npf8(-240.0)  # pad cols: m ~ -1920, never a candidate
    ONES4 = np.zeros((4, 2, 128), npf8)
    ONES4[:, 0, :] = npf8(2.0)
    sqfull = np.full(ncols, -1920.0, np.float32)  # pad cols as in SQ2
    sqfull[:n] = sq32dev
    sq16 = sqfull[lo:].astype(np.float16)
    SQB = np.ascontiguousarray(
        np.broadcast_to(sq16, (128, ncols - lo))
        .reshape(128, NB - act_split, blk)
        .transpose(1, 0, 2)
    )

    starts = [round(k * n / NCORES) for k in range(NCORES + 1)]
    cores, in_maps = [], []
    nthr_all = -(thr_raw + EPS)
    for k in range(NCORES):
        a0, a1 = starts[k], starts[k + 1]
        cnt = a1 - a0
        b = np.zeros((npad, D), np.float32)
        b[:cnt] = -2.0 * emb_s[a0:a1]
        bhiT = b.T.astype(np.float16)
        L = np.stack([bhiT[0:128], bhiT[128:256]])
        nthr_c = np.full(npad, -THR_PAD, np.float32)
        nthr_c[:cnt] = nthr_all[a0:a1]
        NTHR = np.ascontiguousarray(nthr_c.reshape(ntiles, 128).T)
        cores.append(dict(a0=a0, cnt=cnt))
        in_maps.append(
            {"L": L, "R": R, "SQ2": SQ2, "ONES4": ONES4, "SQB": SQB, "NTHR": NTHR}
        )
    return (
        dict(
            emb_s=emb_s,
            labs_s=labs_s,
            sqn=sqn,
            valid=valid,
            cores=cores,
            n=n,
            hp=hp_exact,
            nthr=nthr_all,
        ),
        in_maps,
    )


def host_decode(prep, outs):
    n = prep["n"]
    emb_s, labs_s, sqn, valid, hp, nthr = (
        prep["emb_s"],
        prep["labs_s"],
        prep["sqn"],
        prep["valid"],
        prep["hp"],
        prep["nthr"],
    )
    terms = np.zeros(n, np.float32)
    patch_rows = []
    for k, core in enumerate(prep["cores"]):
        umin_v = outs[k].reshape(-1)
        a0, cnt = core["a0"], core["cnt"]
        xstar = umin_v[:cnt].view(np.float16).astype(np.float32)
        g = a0 + np.arange(cnt)
        vmask = valid[g]
        suspicious = ~(xstar > PATCH_BAND)
        for i in np.nonzero(vmask & suspicious)[0]:
            patch_rows.append(a0 + int(i))
        ok = vmask & ~suspicious
        idx = np.nonzero(ok)[0]
        if idx.size == 0:
            continue
        gg = g[idx]
        mstar = xstar[idx] - nthr[gg]  # x + thr + eps
        hn = np.sqrt(np.maximum(mstar + sqn[gg], 0.0), dtype=np.float32)
        terms[gg] = np.maximum(hp[gg] - hn + MARGIN, np.float32(0.0))

    patch_rows = sorted(set(patch_rows))
    if patch_rows:
        rows = np.array(patch_rows, np.int64)
        sq_rows = (
            sqn[rows][:, None]
            + sqn[None, :]
            - 2.0 * (emb_s[rows] @ emb_s.T).astype(np.float32)
        ).astype(np.float32)
        dist = np.sqrt(np.maximum(sq_rows, 0.0), dtype=np.float32)
        for ridx, gi in enumerate(rows):
            same = labs_s == labs_s[gi]
            pos = same.copy()
            pos[gi] = False
            neg = ~same
            if not pos.any() or not neg.any():
                terms[gi] = 0.0
                continue
            drow = dist[ridx]
            hard_pos = drow[pos].max()
            neg_min = drow[neg].min()
            shn = drow[neg & (drow > hard_pos)]
            hard_neg = shn.min() if shn.size else neg_min
            terms[gi] = max(hard_pos - hard_neg + MARGIN, np.float32(0.0))

    cnt_valid = valid.sum()
    if cnt_valid > 0:
        return np.float32(terms.sum(dtype=np.float32) / max(cnt_valid, 1))
    return np.float32(0.0)


# ---------------------------------------------------------------- entry
def kernel(embeddings, tags, confidences):
    from concourse.bass_utils import run_bass_kernel_spmd

    dims = FULL_DIMS
    nc = _get_nc("full", dims)
    prep, in_maps = host_prep(embeddings, tags, confidences, dims)
    res = run_bass_kernel_spmd(nc, in_maps, list(range(NCORES)))
    outs = [
        np.ascontiguousarray(res.results[k]["umin_out"]).astype(np.uint16)
        for k in range(NCORES)
    ]
    loss = host_decode(prep, outs)
    return np.array(loss, dtype=np.float32)
